# revision 3
# baseline (speedup 1.0000x reference)
"""Trainium2 Bass kernel for nn_KA_attention (KA-style sin-basis attention).

Math (per reference):
  For branch (x, coef):  row r = ((h*64+p)*64+d)*64+o  (R = 2,097,152 rows)
    xt[r] = x[b, p mod 8, d, o]   (derived: r mod N with o-major tiling)
    F[b,r] = scale_sp[r] * sum_f coef[r,f]*sin(grid[f]*xt) + scale_base[r]*silu(xt)
  y[b,h,p,o] = sum_d F_q[...] + sum_d F_k[...] + bias[h,p,o];  softmax over o.

Sharding: contiguous R-chunks across 8 cores == one head per core.
Everything (d-reduction, softmax) is core-local; no collectives.

Device strategy per core:
  - coef rows (+ scale_base as a 9th column) are streamed as [128=(parity,d),
    pair, (o,9)] tiles; a custom fused DVE op (multiply + prefix-scan)
    produces running sums at 1 elem/cycle; segment sums are recovered on the
    idle TensorEngine: per-parity column-sum matmuls (masks) followed by a
    bidiagonal difference matmul, which simultaneously performs the
    d-reduction. A PE transpose + short softmax finishes.
  - sin arguments are host-range-reduced (HW Sin table covers only ~±3.55).
"""
import os
import numpy as np

from concourse import bass, bacc, tile, mybir
from concourse.bass_utils import run_bass_kernel_spmd
from concourse import dve_ops
from concourse.dve_spec import Spec, Src0, Src1, scan, AluOp, lower, _has_src1
from concourse.dve_uop import DveOpSpec

F32 = mybir.dt.float32
AF = mybir.ActivationFunctionType

BATCH, HEADS, PATCHES, DIM, OUT_DIM, NUM_F = 4, 8, 64, 64, 64, 8
N = HEADS * PATCHES * DIM          # 32768
R = N * OUT_DIM                    # 2097152
RPC = R // 8                       # rows per core = 262144
NSEG = NUM_F + 1                   # 8 sin coefs + 1 silu/scale_base column

SEGDOT_NAME = "SEGDOT_SCAN_ANT"


def _segdot_ref(in0, in1, s0, s1, imm2):
    p = in0.shape[0]
    prod = (np.asarray(in0, np.float32) * np.asarray(in1, np.float32)).reshape(p, -1)
    acc = np.add.accumulate(prod.astype(np.float32), axis=1, dtype=np.float32)
    return acc.reshape(in0.shape).astype(np.float32)


def _register_segdot():
    for op in dve_ops.OPS:
        if op.name == SEGDOT_NAME:
            return op
    spec = Spec(body=scan(AluOp.ADD, Src0 * Src1), reference=_segdot_ref)
    opcode = max(dve_ops._SUB_OPCODE_FOR_NAME.values()) + 1
    dve_ops._SUB_OPCODE_FOR_NAME[SEGDOT_NAME] = opcode
    shas = {}
    for ver in ("v3", "v4"):
        s = DveOpSpec(name=SEGDOT_NAME, opcode=opcode,
                      uops=lower(spec, ver=ver), rd1_en=_has_src1(spec))
        shas[ver] = s.sha(ver)
    op = dve_ops.DveOp(SEGDOT_NAME, spec, subdim=False, uops_sha=shas)
    dve_ops.OPS.append(op)
    dve_ops.CUSTOM_DVE_SPECS[SEGDOT_NAME] = spec
    return op


def build_program():
    op = _register_segdot()
    nc = bacc.Bacc("TRN2", target_bir_lowering=False, debug=False, num_devices=8)

    cqi = nc.dram_tensor("cqi", [128, 32, 9 * DIM], F32, kind="ExternalInput").ap()
    cki = nc.dram_tensor("cki", [128, 32, 9 * DIM], F32, kind="ExternalInput").ap()
    xsq = nc.dram_tensor("xsq", [128, NUM_F, 1024], F32, kind="ExternalInput").ap()
    xsk = nc.dram_tensor("xsk", [128, NUM_F, 1024], F32, kind="ExternalInput").ap()
    xlq = nc.dram_tensor("xlq", [128, 1024], F32, kind="ExternalInput").ap()
    xlk = nc.dram_tensor("xlk", [128, 1024], F32, kind="ExternalInput").ap()
    bias2 = nc.dram_tensor("bias2", [128, 128], F32, kind="ExternalInput").ap()
    masks = nc.dram_tensor("masks", [128, 2], F32, kind="ExternalInput").ap()
    ident = nc.dram_tensor("ident", [128, 128], F32, kind="ExternalInput").ap()
    yraw = nc.dram_tensor("yraw", [128, 128], F32, kind="ExternalOutput").ap()

    with tile.TileContext(nc) as tc:
        with (
            tc.tile_pool(name="const", bufs=1) as cpool,
            tc.tile_pool(name="xin", bufs=1) as xpool,
            tc.tile_pool(name="sintab", bufs=1) as spool,
            tc.tile_pool(name="cf", bufs=3) as cfpool,
            tc.tile_pool(name="pref", bufs=3) as ppool,
            tc.tile_pool(name="small", bufs=1) as mpool,
            tc.tile_pool(name="psum", bufs=1, space="PSUM") as psum,
        ):
            tm = cpool.tile([128, 2], F32)
            ti = cpool.tile([128, 128], F32)
            tb2 = cpool.tile([128, 128], F32)
            nc.sync.dma_start(tm[:], masks)
            nc.sync.dma_start(ti[:], ident)
            nc.sync.dma_start(tb2[:], bias2)

            # --- basis tables: sin((f+1)x) for f<8 (host-reduced args), silu(x)
            tsin = {}
            for br, (xs_d, xl_d) in (("q", (xsq, xlq)), ("k", (xsk, xlk))):
                ts = spool.tile([128, 4, BATCH, DIM * NSEG], F32, tag=f"sin{br}")
                tsin[br] = ts
                txs = xpool.tile([128, NUM_F, 1024], F32, tag="xs")
                txl = xpool.tile([128, 1024], F32, tag="xl")
                nc.sync.dma_start(txs[:], xs_d)
                nc.sync.dma_start(txl[:], xl_d)
                tsr = ts[:].rearrange("p a b (o n) -> p a b o n", n=NSEG)
                for f in range(NUM_F):
                    nc.scalar.activation(
                        tsr[:, :, :, :, f],
                        txs[:, f, :].rearrange("p (a b o) -> p a b o", a=4, b=BATCH),
                        AF.Sin)
                nc.scalar.activation(
                    tsr[:, :, :, :, NUM_F],
                    txl[:].rearrange("p (a b o) -> p a b o", a=4, b=BATCH),
                    AF.Silu)

            # --- main loop: fused multiply+scan, then segment sums via PE
            mpsum = psum.tile([128, 256], F32)
            for bi, (br, coef_d) in enumerate((("q", cqi), ("k", cki))):
                for g in range(8):
                    cf = cfpool.tile([128, 4, 9 * DIM], F32, tag="cf")
                    nc.sync.dma_start(cf[:], coef_d[:, 4 * g:4 * g + 4, :])
                    for b in range(BATCH):
                        pt = ppool.tile([128, 2, 18 * DIM], F32, tag="pref")
                        for j in range(2):
                            nc.vector._custom_dve(
                                op,
                                out=pt[:, j, :].rearrange(
                                    "p (s n) -> p s n", n=9 * DIM),
                                in0=cf[:, 2 * j:2 * j + 2, :],
                                in1=tsin[br][:, 2 * j:2 * j + 2, b, :])
                        qv = pt[:].rearrange(
                            "p i (seg n) -> p i seg n", n=NSEG)[:, :, :, NSEG - 1]
                        fd = ppool.tile([128, 2, 128], F32, tag="fdif")
                        nc.vector.tensor_tensor(
                            fd[:, :, 1:], qv[:, :, 1:], qv[:, :, :-1],
                            op=mybir.AluOpType.subtract)
                        nc.vector.tensor_copy(fd[:, :, 0:1], qv[:, :, 0:1])
                        for j in range(2):
                            col = 2 * (bi * 64 + b * 16 + g * 2 + j)
                            nc.tensor.matmul(
                                mpsum[:, col:col + 2], fd[:, j, :], tm[:],
                                start=True, stop=True)

            # --- tail: diff (d-sum already in M), +bias, transpose, softmax
            ms = mpool.tile([128, 256], F32)
            nc.scalar.copy(ms[:], mpsum[:])
            f2 = mpool.tile([128, 128], F32)
            nc.vector.tensor_tensor(f2[:], ms[:, 0:128], ms[:, 128:256],
                                    op=mybir.AluOpType.add)
            f3 = mpool.tile([128, 128], F32)
            nc.vector.tensor_tensor(f3[:], f2[:], tb2[:], op=mybir.AluOpType.add)
            tps = psum.tile([128, 128], F32)
            nc.tensor.transpose(tps[:], f3[:], ti[:])

            yout = mpool.tile([128, 2, 64], F32)
            for half in range(2):
                sub = tps[:, 64 * half:64 * half + 64]
                mxn = mpool.tile([128, 1], F32, tag=f"mx{half}")
                nc.vector.tensor_reduce(mxn[:], sub, axis=mybir.AxisListType.X,
                                        op=mybir.AluOpType.max, negate=True)
                ex = mpool.tile([128, 64], F32, tag=f"ex{half}")
                nc.scalar.activation(ex[:], sub, AF.Exp, bias=mxn[:])
                sm = mpool.tile([128, 1], F32, tag=f"sm{half}")
                nc.vector.tensor_reduce(sm[:], ex[:], axis=mybir.AxisListType.X,
                                        op=mybir.AluOpType.add)
                rc = mpool.tile([128, 1], F32, tag=f"rc{half}")
                nc.vector.reciprocal(rc[:], sm[:])
                nc.vector.scalar_tensor_tensor(
                    out=yout[:, half, :], in0=ex[:], scalar=rc[:], in1=ex[:],
                    op0=mybir.AluOpType.mult, op1=mybir.AluOpType.bypass)

            nc.sync.dma_start(yraw, yout[:].rearrange("p h o -> p (h o)"))

    nc.compile()
    return nc


_NC_CACHE = None


def _get_nc():
    global _NC_CACHE
    if _NC_CACHE is None:
        _NC_CACHE = build_program()
    return _NC_CACHE


def host_prep(q, k, grid, coef_q, coef_k, bias_w, scale_base, scale_sp):
    """Build per-core input maps (all layout/bookkeeping, no O(R) math beyond
    an optional exact scale_sp fold)."""
    q = np.asarray(q, np.float32)
    k = np.asarray(k, np.float32)
    grid = np.asarray(grid, np.float32)
    coef_q = np.asarray(coef_q, np.float32)
    coef_k = np.asarray(coef_k, np.float32)
    bias_w = np.asarray(bias_w, np.float32)
    scale_base = np.asarray(scale_base, np.float32)
    scale_sp = np.asarray(scale_sp, np.float32)

    if not np.all(scale_sp == 1.0):
        coef_q = coef_q * scale_sp[:, None]
        coef_k = coef_k * scale_sp[:, None]

    # X tables, shared by all cores: X[(parity,d), pp, b, o] = x[b, pp*2+parity, d, o]
    def xtab(x):
        xh = x.transpose(1, 2, 0, 3).reshape(4, 2, DIM, BATCH, OUT_DIM)
        X = np.ascontiguousarray(
            xh.transpose(1, 2, 0, 3, 4).reshape(128, 1024)).astype(np.float32)
        args = grid[:, None, None] * X[None]          # [8, 128, 1024]
        red = np.mod(args + np.pi, 2 * np.float32(np.pi)) - np.float32(np.pi)
        xs = np.ascontiguousarray(red.transpose(1, 0, 2)).astype(np.float32)
        return xs, X

    xsq, xlq = xtab(q)
    xsk, xlk = xtab(k)

    masks = np.zeros((128, 2), np.float32)
    masks[:64, 0] = 1.0
    masks[64:, 1] = 1.0
    ident = np.eye(128, dtype=np.float32)

    bias_hp = bias_w[0].reshape(HEADS, PATCHES, OUT_DIM)

    def interleave(coef_slice, sb_slice):
        cr = coef_slice.reshape(32, 2, DIM, OUT_DIM, NUM_F)
        sbr = sb_slice.reshape(32, 2, DIM, OUT_DIM, 1)
        ci = np.concatenate([cr, sbr], axis=4)        # [pair, parity, d, o, 9]
        return np.ascontiguousarray(
            ci.transpose(1, 2, 0, 3, 4).reshape(128, 32, 9 * DIM))

    in_maps = []
    for c in range(8):
        sl = slice(c * RPC, (c + 1) * RPC)
        cqi = interleave(coef_q[sl], scale_base[sl])
        cki = interleave(coef_k[sl], scale_base[sl])
        b2 = np.zeros((128, 128), np.float32)
        for g in range(8):
            for j in range(2):
                for pl in range(2):
                    for par in range(2):
                        p = 8 * g + 4 * j + 2 * pl + par
                        for b in range(BATCH):
                            col = 2 * (b * 16 + g * 2 + j) + par
                            b2[pl * 64:pl * 64 + 64, col] = bias_hp[c, p]
        in_maps.append({
            "cqi": cqi, "cki": cki, "xsq": xsq, "xsk": xsk,
            "xlq": xlq, "xlk": xlk, "bias2": b2, "masks": masks,
            "ident": ident,
        })
    return in_maps


def host_post(results):
    """[128,128] yraw per core -> full [4, 8, 64, 64]."""
    y = np.empty((BATCH, HEADS, PATCHES, OUT_DIM), np.float32)
    for c in range(8):
        yr = results[c]["yraw"].reshape(BATCH, 8, 2, 2, 2, OUT_DIM)
        y[:, c] = yr.transpose(0, 1, 2, 4, 3, 5).reshape(BATCH, PATCHES, OUT_DIM)
    return y


def kernel(q, k, grid, coef_q, coef_k, bias_w, scale_base, scale_sp):
    nc = _get_nc()
    in_maps = host_prep(q, k, grid, coef_q, coef_k, bias_w, scale_base, scale_sp)
    res = run_bass_kernel_spmd(nc, in_maps, list(range(8)))
    return host_post(res.results)
